# revision 15
# baseline (speedup 1.0000x reference)
"""Trainium2 Bass kernel v3 for nn_AsynBaseStem: dense masked 7x7 conv + BN +
ReLU + 3x3/2 maxpool, 8-core data-parallel over output row bands.

v3 changes vs v2 (80us baseline):
  - Even/odd matmul split: each x-tile runs separate stride-2 rhs matmuls for
    even columns (PSUM tile E) and odd columns (PSUM tile O), so the ACT
    eviction of evens is a contiguous PSUM read and every SBUF-side DVE op
    stays 4B-aligned for the 2x bf16 perf mode (x-advance 1020, v-advance 510).
  - ReLU folded into the final vertical-pool scalar_tensor_tensor
    (out = (s01 max 0) max m_row), removing all ACT ReLU passes; ACT only
    drains the even PSUM columns.
  - Output DMA via HWDGE (sync queue) instead of SWDGE/gpsimd - kills the
    ~4us Q7 descriptor-emission tail.
  - Table chunk loads split across the sync AND scalar HWDGE queues so
    triggers issue in parallel and the first matmul starts ~1.5us in.
  - Optional gpsimd offload of one class's vertical pool stage.

Pipeline per tile (1020 x-cols) per class: PE writes E[128,512]/O[128,512]
fp32 PSUM (3 accumulating passes x 2 sub-band halves); ACT copies E->evs
(bf16); DVE tb = max(O, evs[v+1]) (1x, PSUM), m = max(tb, evs[v]) (2x);
vertical 3:2 pool s01 = max(m2p, m2p+1), pooled = (s01 max 0) max m2p+2.
"""
import numpy as np
import ml_dtypes
from contextlib import ExitStack

H = W = 640
CIN, COUT = 3, 64
KK, PAD = 7, 3
NCORES = 8
NEG = -1.0e9
BN_EPS = 1e-5

WP = 648                  # row pitch in dense x space
GROWS = 41                # dense rows per partition-group table
GCOLS = GROWS * WP        # 26568
GPAD = GCOLS + 8
BROWS = 21                # dense rows per sub-band
NPOOL = 10                # pooled rows per sub-band
MPITCH = 324              # m-buffer row pitch (v space)
BX0 = 20 * WP             # upper sub-band band-local x origin (12960)

XT = 1020                 # x advance per tile
VT = 510                  # v advance per tile
NFULL = 13                # full tiles; tail tile has NE=172/NO=171/MW=171
VBUF = 6804               # evs/tb/m buffer cols (21*324; max index used 6801)

VERT_ON_GPSIMD = (False, False)  # TRN2 Pool engine rejects TensorTensor


def _build_bass(with_bias):
    import concourse.bass as bass
    import concourse.mybir as mybir
    import concourse.tile as tile
    from concourse import bacc

    fp32 = mybir.dt.float32
    bf16 = mybir.dt.bfloat16
    MAX = mybir.AluOpType.max

    nc = bacc.Bacc()
    tbl_ext = nc.declare_dram_parameter("tbl", [128, GPAD], bf16, isOutput=False)
    wt_ext = nc.declare_dram_parameter("wt", [128, 192], bf16, isOutput=False)
    if with_bias:
        bias_ext = nc.declare_dram_parameter("bias", [128, 1], fp32,
                                             isOutput=False)
    out_ext = nc.declare_dram_parameter("out", [128, 2 * NPOOL * 320], bf16,
                                        isOutput=True)

    with ExitStack() as ctx:
        tc = ctx.enter_context(tile.TileContext(nc))
        cpool = ctx.enter_context(tc.tile_pool(name="const", bufs=1))
        psp = ctx.enter_context(tc.tile_pool(name="ps", bufs=2, space="PSUM"))

        # table chunk loads: lower-half rows on the sync queue, upper-half on
        # the scalar queue (both HWDGE) so triggers issue concurrently and
        # early tiles unblock fast; first chunks are tiny so tile 0 starts
        # ASAP.
        tbl = cpool.tile([128, GPAD], bf16)
        wt = cpool.tile([128, 192], bf16)
        if with_bias:
            bias = cpool.tile([128, 1], fp32)
        sync_chunks = [(0, 1), (1, 4), (4, 8), (8, 14), (14, 20)]
        act_chunks = [(20, 21), (21, 24), (24, 28), (28, 34), (34, 41)]
        first = True
        for (r0s, r1s), (r0a, r1a) in zip(sync_chunks, act_chunks):
            a, b = r0s * WP, r1s * WP
            nc.sync.dma_start(tbl[:, a:b], tbl_ext[:, a:b])
            if first:
                nc.scalar.dma_start(wt[:], wt_ext[:])
                if with_bias:
                    nc.scalar.dma_start(bias[:], bias_ext[:])
                first = False
            a, b = r0a * WP, (r1a * WP if r1a < GROWS else GPAD)
            nc.scalar.dma_start(tbl[:, a:b], tbl_ext[:, a:b])

        tbv = [cpool.tile([128, VBUF], bf16, name=f"tbv{c}") for c in range(2)]
        mbv = [cpool.tile([128, VBUF], bf16, name=f"mbv{c}") for c in range(2)]
        s01 = [cpool.tile([128, NPOOL * 320], bf16, name=f"s01_{c}")
               for c in range(2)]
        pooled = [cpool.tile([128, NPOOL * 320], bf16, name=f"pool{c}")
                  for c in range(2)]
        mb3 = [a.rearrange("p (a b) -> p a b", b=MPITCH) for a in mbv]
        s013 = [a.rearrange("p (a b) -> p a b", b=320) for a in s01]
        pooled3 = [a.rearrange("p (a b) -> p a b", b=320) for a in pooled]

        pdone = [0, 0]   # pooled rows emitted per class
        odone = [0, 0]   # pooled rows DMA'd out per class

        def emit_vert(cls, upto):
            p0 = pdone[cls]
            while p0 < upto:
                n = min(5, upto - p0)
                nc.vector.tensor_tensor(
                    out=s013[cls][:, p0:p0 + n, :],
                    in0=mb3[cls][:, 2 * p0:2 * p0 + 2 * n:2, 0:320],
                    in1=mb3[cls][:, 2 * p0 + 1:2 * p0 + 2 * n:2, 0:320],
                    op=MAX)
                if with_bias:
                    nc.vector.tensor_tensor(
                        out=pooled3[cls][:, p0:p0 + n, :],
                        in0=s013[cls][:, p0:p0 + n, :],
                        in1=mb3[cls][:, 2 * p0 + 2:2 * p0 + 2 * n + 1:2, 0:320],
                        op=MAX)
                else:
                    # fused ReLU: (s01 max 0) max m[2p+2]
                    nc.vector.scalar_tensor_tensor(
                        out=pooled3[cls][:, p0:p0 + n, :],
                        in0=s013[cls][:, p0:p0 + n, :],
                        scalar=0.0,
                        in1=mb3[cls][:, 2 * p0 + 2:2 * p0 + 2 * n + 1:2, 0:320],
                        op0=MAX, op1=MAX)
                p0 += n
            pdone[cls] = upto

        def emit_out(cls, upto):
            a = odone[cls] * 320
            b = upto * 320
            if with_bias:
                nc.scalar.activation(pooled[cls][:, a:b], pooled[cls][:, a:b],
                                     mybir.ActivationFunctionType.Relu,
                                     bias=bias[:, 0:1])
            # final chunks split across both HWDGE queues to parallelize the
            # tail transfers
            eng = nc.scalar if (upto == NPOOL and cls == 1) else nc.sync
            eng.dma_start(
                out_ext[:, cls * NPOOL * 320 + a:cls * NPOOL * 320 + b],
                pooled[cls][:, a:b])
            odone[cls] = upto

        for Wt in range(NFULL + 1):
            full = Wt < NFULL
            NE = 511 if full else 172
            NO = 511 if full else 171
            MW = 510 if full else 171
            v0 = VT * Wt
            x0 = XT * Wt
            psE = [psp.tile([128, 512], fp32, tag=f"E{c}", name=f"E{c}_{Wt}")
                   for c in range(2)]
            psO = [psp.tile([128, 512], fp32, tag=f"O{c}", name=f"O{c}_{Wt}")
                   for c in range(2)]
            for p in range(3):
                st, sp = (p == 0), (p == 2)
                for half in range(2):
                    hx0 = x0 + (BX0 if half else 0) + 3 * p
                    ob = 64 * half
                    for cls in range(2):
                        gp = 64 * cls
                        w = wt[gp:gp + 64, 64 * p:64 * p + 64]
                        nc.tensor.matmul(
                            psE[cls][ob:ob + 64, 0:NE], w,
                            tbl[gp:gp + 64, hx0:hx0 + 2 * NE:2],
                            start=st, stop=sp)
                        nc.tensor.matmul(
                            psO[cls][ob:ob + 64, 0:NO], w,
                            tbl[gp:gp + 64, hx0 + 1:hx0 + 1 + 2 * NO:2],
                            start=st, stop=sp)
            for cls in range(2):
                # evens land straight in the m-buffer (skip elem 0 after
                # tile 0: duplicate of the previous tile's last elem)
                if Wt == 0:
                    nc.scalar.copy(mbv[cls][:, 0:NE], psE[cls][:, 0:NE])
                else:
                    nc.scalar.copy(mbv[cls][:, v0 + 1:v0 + NE],
                                   psE[cls][:, 1:NE])
                # tb = max(O, E[v+1])  (1x: PSUM operand)
                nc.vector.tensor_tensor(
                    out=tbv[cls][:, v0:v0 + MW],
                    in0=psO[cls][:, 0:MW],
                    in1=mbv[cls][:, v0 + 1:v0 + 1 + MW],
                    op=MAX)
                # m = max(E[v], tb)  (2x: flat bf16 SBUF)
                nc.vector.tensor_tensor(
                    out=mbv[cls][:, v0:v0 + MW],
                    in0=tbv[cls][:, v0:v0 + MW],
                    in1=mbv[cls][:, v0:v0 + MW],
                    op=MAX)
                # pooled rows ready: (2p+2)*324 + 321 <= mdone
                mdone = v0 + MW
                pready = min(NPOOL, max(0, (mdone - 969) // 648 + 1))
                if pready - pdone[cls] >= 2 or (pready == NPOOL
                                                and pready > pdone[cls]):
                    emit_vert(cls, pready)
                    for tgt in (3, 6, 9, 10):
                        if pdone[cls] >= tgt and odone[cls] < tgt:
                            emit_out(cls, tgt)
        for cls in range(2):
            if pdone[cls] < NPOOL:
                emit_vert(cls, NPOOL)
            if odone[cls] < NPOOL:
                emit_out(cls, NPOOL)

    nc.finalize()
    return nc


_NC_CACHE = {}


def _get_nc(with_bias=False):
    if with_bias not in _NC_CACHE:
        _NC_CACHE[with_bias] = _build_bass(with_bias)
    return _NC_CACHE[with_bias]


def build_in_maps(update_location, feature_map, weight, gamma, beta,
                  running_mean, running_var):
    fm = np.asarray(feature_map, np.float32)
    loc = np.asarray(update_location).astype(np.int64)
    wt_ = np.asarray(weight, np.float32)
    gam = np.asarray(gamma, np.float32)
    bet = np.asarray(beta, np.float32)
    mu = np.asarray(running_mean, np.float32)
    var = np.asarray(running_var, np.float32)

    inv = gam / np.sqrt(var + BN_EPS)
    wf = wt_ * inv[None, None, None, :]          # [7,7,3,64]
    bias = bet - mu * inv                        # [64]

    # fm_pad with extra bottom rows so group1 of core 7 stays in bounds
    fmp = np.zeros((H + 2 * PAD + 2, W + 2 * PAD, CIN), np.float32)
    fmp[PAD:PAD + H, PAD:PAD + W] = fm

    # inactive flag per output pixel; cols >= 640 and rows >= 640 inactive
    flag = np.ones((H + 2, W + 6), np.float32)
    flag[loc[:, 0], loc[:, 1]] = 0.0
    flag[:, W:] = 1.0
    flag[H:, :] = 1.0

    # weight rows [128, 192]: pass p block = W'[i, j'+3p, ch, :]
    wrows = np.zeros((64, 192), np.float32)
    for jp in range(3):
        for i in range(KK):
            for ch in range(CIN):
                row = jp * 21 + i * 3 + ch
                for p in range(3):
                    j = jp + 3 * p
                    if j <= 6:
                        wrows[row, 64 * p:64 * p + 64] = wf[i, j, ch]
    wrows[63, 0:64] = NEG
    wt128 = np.concatenate([wrows, wrows], axis=0).astype(ml_dtypes.bfloat16)
    with_bias = bool(np.any(bias != 0.0))
    bias128 = np.concatenate([bias, bias]).reshape(128, 1).astype(np.float32)

    in_maps = []
    for k in range(NCORES):
        tblk = np.zeros((128, GPAD), ml_dtypes.bfloat16)
        for g in range(2):
            r0 = 80 * k + 40 * g
            S = np.zeros((64, GROWS, WP), np.float32)
            for jp in range(3):
                for i in range(KK):
                    for ch in range(CIN):
                        S[jp * 21 + i * 3 + ch, :, 0:W + 2 * PAD - jp] = \
                            fmp[r0 + i:r0 + i + GROWS, jp:, ch]
            S[63, :, 0:W + 6] = flag[r0:r0 + GROWS, :]
            tblk[64 * g:64 * g + 64, 0:GCOLS] = \
                S.reshape(64, GCOLS).astype(ml_dtypes.bfloat16)
        m = {"tbl": tblk, "wt": wt128}
        if with_bias:
            m["bias"] = bias128
        in_maps.append(m)
    return in_maps, with_bias


def kernel(update_location, feature_map, weight, gamma, beta, running_mean,
           running_var):
    from concourse.bass_utils import run_bass_kernel_spmd

    in_maps, with_bias = build_in_maps(
        update_location, feature_map, weight, gamma, beta, running_mean,
        running_var)
    nc = _get_nc(with_bias)
    res = run_bass_kernel_spmd(nc, in_maps, core_ids=list(range(NCORES)))
    # per-core out [128, 6400] bf16: [part, cls*3200 + p*320 + q];
    # partitions 0-63 = channels of the lower sub-band, 64-127 = upper.
    out = np.zeros((NCORES * 40, 320, COUT), np.float32)
    for k in range(NCORES):
        o = np.asarray(res.results[k]["out"], dtype=np.float32)
        o = o.reshape(2, 64, 2, NPOOL, 320)      # [half, ch, cls, p, q]
        for cls in range(2):
            for half in range(2):
                band = 2 * cls + half
                r = 40 * k + 10 * band
                out[r:r + NPOOL] = o[half, :, cls].transpose(1, 2, 0)
    return np.ascontiguousarray(out[:319, :319, :]).astype(np.float32)


# revision 19
# speedup vs baseline: 1.0915x; 1.0915x over previous
"""Trainium2 Bass kernel v3 for nn_AsynBaseStem: dense masked 7x7 conv + BN +
ReLU + 3x3/2 maxpool, 8-core data-parallel over output row bands.

v3 changes vs v2 (80us baseline):
  - Even/odd matmul split: each x-tile runs separate stride-2 rhs matmuls for
    even columns (PSUM tile E) and odd columns (PSUM tile O), so the ACT
    eviction of evens is a contiguous PSUM read and every SBUF-side DVE op
    stays 4B-aligned for the 2x bf16 perf mode (x-advance 1020, v-advance 510).
  - ReLU folded into the final vertical-pool scalar_tensor_tensor
    (out = (s01 max 0) max m_row), removing all ACT ReLU passes; ACT only
    drains the even PSUM columns.
  - Output DMA via HWDGE (sync queue) instead of SWDGE/gpsimd - kills the
    ~4us Q7 descriptor-emission tail.
  - Table chunk loads split across the sync AND scalar HWDGE queues so
    triggers issue in parallel and the first matmul starts ~1.5us in.
  - Optional gpsimd offload of one class's vertical pool stage.

Pipeline per tile (1020 x-cols) per class: PE writes E[128,512]/O[128,512]
fp32 PSUM (3 accumulating passes x 2 sub-band halves); ACT copies E->evs
(bf16); DVE tb = max(O, evs[v+1]) (1x, PSUM), m = max(tb, evs[v]) (2x);
vertical 3:2 pool s01 = max(m2p, m2p+1), pooled = (s01 max 0) max m2p+2.
"""
import numpy as np
import ml_dtypes
from contextlib import ExitStack

H = W = 640
CIN, COUT = 3, 64
KK, PAD = 7, 3
NCORES = 8
NEG = -1.0e9
BN_EPS = 1e-5

WP = 648                  # row pitch in dense x space
GROWS = 41                # dense rows per partition-group table
GCOLS = GROWS * WP        # 26568
GPAD = GCOLS + 8
BROWS = 21                # dense rows per sub-band
NPOOL = 10                # pooled rows per sub-band
MPITCH = 324              # m-buffer row pitch (v space)
BX0 = 20 * WP             # upper sub-band band-local x origin (12960)

XT = 1020                 # x advance per tile
VT = 510                  # v advance per tile
NFULL = 13                # full tiles; tail tile has NE=172/NO=171/MW=171
VBUF = 6804               # evs/tb/m buffer cols (21*324; max index used 6801)

VERT_ON_GPSIMD = (False, False)  # TRN2 Pool engine rejects TensorTensor


def _build_bass(with_bias):
    import concourse.bass as bass
    import concourse.mybir as mybir
    import concourse.tile as tile
    from concourse import bacc

    fp32 = mybir.dt.float32
    bf16 = mybir.dt.bfloat16
    MAX = mybir.AluOpType.max

    nc = bacc.Bacc()
    tbl_ext = nc.declare_dram_parameter("tbl", [128, GPAD], bf16, isOutput=False)
    wt_ext = nc.declare_dram_parameter("wt", [128, 192], bf16, isOutput=False)
    if with_bias:
        bias_ext = nc.declare_dram_parameter("bias", [128, 1], fp32,
                                             isOutput=False)
    out_ext = nc.declare_dram_parameter("out", [128, 2 * NPOOL * 320], bf16,
                                        isOutput=True)

    with ExitStack() as ctx:
        tc = ctx.enter_context(tile.TileContext(nc))
        cpool = ctx.enter_context(tc.tile_pool(name="const", bufs=1))
        psp = ctx.enter_context(tc.tile_pool(name="ps", bufs=2, space="PSUM"))

        # table chunk loads: lower-half rows on the sync queue, upper-half on
        # the scalar queue (both HWDGE) so triggers issue concurrently and
        # early tiles unblock fast; first chunks are tiny so tile 0 starts
        # ASAP.
        tbl = cpool.tile([128, GPAD], bf16)
        wt = cpool.tile([128, 192], bf16)
        if with_bias:
            bias = cpool.tile([128, 1], fp32)
        sync_chunks = [(0, 2), (2, 5), (5, 9), (9, 14), (14, 20)]
        act_chunks = [(20, 22), (22, 25), (25, 29), (29, 35), (35, 41)]
        first = True
        for (r0s, r1s), (r0a, r1a) in zip(sync_chunks, act_chunks):
            a, b = r0s * WP, r1s * WP
            nc.sync.dma_start(tbl[:, a:b], tbl_ext[:, a:b])
            if first:
                nc.scalar.dma_start(wt[:], wt_ext[:])
                if with_bias:
                    nc.scalar.dma_start(bias[:], bias_ext[:])
                first = False
            a, b = r0a * WP, (r1a * WP if r1a < GROWS else GPAD)
            nc.scalar.dma_start(tbl[:, a:b], tbl_ext[:, a:b])

        evs = [cpool.tile([128, VBUF], bf16, name=f"evs{c}") for c in range(2)]
        tbv = [cpool.tile([128, VBUF], bf16, name=f"tbv{c}") for c in range(2)]
        mbv = [cpool.tile([128, VBUF], bf16, name=f"mbv{c}") for c in range(2)]
        s01 = [cpool.tile([128, NPOOL * 320], bf16, name=f"s01_{c}")
               for c in range(2)]
        pooled = [cpool.tile([128, NPOOL * 320], bf16, name=f"pool{c}")
                  for c in range(2)]
        mb3 = [a.rearrange("p (a b) -> p a b", b=MPITCH) for a in mbv]
        s013 = [a.rearrange("p (a b) -> p a b", b=320) for a in s01]
        pooled3 = [a.rearrange("p (a b) -> p a b", b=320) for a in pooled]

        pdone = [0, 0]   # pooled rows emitted per class
        odone = [0, 0]   # pooled rows DMA'd out per class

        def emit_vert(cls, upto):
            p0 = pdone[cls]
            while p0 < upto:
                n = min(5, upto - p0)
                nc.vector.tensor_tensor(
                    out=s013[cls][:, p0:p0 + n, :],
                    in0=mb3[cls][:, 2 * p0:2 * p0 + 2 * n:2, 0:320],
                    in1=mb3[cls][:, 2 * p0 + 1:2 * p0 + 2 * n:2, 0:320],
                    op=MAX)
                if with_bias:
                    nc.vector.tensor_tensor(
                        out=pooled3[cls][:, p0:p0 + n, :],
                        in0=s013[cls][:, p0:p0 + n, :],
                        in1=mb3[cls][:, 2 * p0 + 2:2 * p0 + 2 * n + 1:2, 0:320],
                        op=MAX)
                else:
                    # fused ReLU: (s01 max 0) max m[2p+2]
                    nc.vector.scalar_tensor_tensor(
                        out=pooled3[cls][:, p0:p0 + n, :],
                        in0=s013[cls][:, p0:p0 + n, :],
                        scalar=0.0,
                        in1=mb3[cls][:, 2 * p0 + 2:2 * p0 + 2 * n + 1:2, 0:320],
                        op0=MAX, op1=MAX)
                p0 += n
            pdone[cls] = upto

        def emit_out(cls, upto):
            a = odone[cls] * 320
            b = upto * 320
            if with_bias:
                nc.scalar.activation(pooled[cls][:, a:b], pooled[cls][:, a:b],
                                     mybir.ActivationFunctionType.Relu,
                                     bias=bias[:, 0:1])
            # final chunks split across both HWDGE queues to parallelize the
            # tail transfers
            eng = nc.scalar if (upto == NPOOL and cls == 1) else nc.sync
            eng.dma_start(
                out_ext[:, cls * NPOOL * 320 + a:cls * NPOOL * 320 + b],
                pooled[cls][:, a:b])
            odone[cls] = upto

        for Wt in range(NFULL + 1):
            full = Wt < NFULL
            NE = 511 if full else 172
            NO = 511 if full else 171
            MW = 510 if full else 171
            v0 = VT * Wt
            x0 = XT * Wt
            psE = [psp.tile([128, 512], fp32, tag=f"E{c}", name=f"E{c}_{Wt}")
                   for c in range(2)]
            psO = [psp.tile([128, 512], fp32, tag=f"O{c}", name=f"O{c}_{Wt}")
                   for c in range(2)]
            for p in range(3):
                st, sp = (p == 0), (p == 2)
                for half in range(2):
                    hx0 = x0 + (BX0 if half else 0) + 3 * p
                    ob = 64 * half
                    for cls in range(2):
                        gp = 64 * cls
                        w = wt[gp:gp + 64, 64 * p:64 * p + 64]
                        nc.tensor.matmul(
                            psE[cls][ob:ob + 64, 0:NE], w,
                            tbl[gp:gp + 64, hx0:hx0 + 2 * NE:2],
                            start=st, stop=sp)
                        nc.tensor.matmul(
                            psO[cls][ob:ob + 64, 0:NO], w,
                            tbl[gp:gp + 64, hx0 + 1:hx0 + 1 + 2 * NO:2],
                            start=st, stop=sp)
            for cls in range(2):
                # evens -> evs (skip elem 0 after tile 0: duplicate of the
                # previous tile's last elem; avoids a cross-tile WAR stall)
                if Wt == 0:
                    nc.scalar.copy(evs[cls][:, 0:NE], psE[cls][:, 0:NE])
                else:
                    nc.scalar.copy(evs[cls][:, v0 + 1:v0 + NE],
                                   psE[cls][:, 1:NE])
                # tb = max(O, E[v+1])  (1x: PSUM operand)
                nc.vector.tensor_tensor(
                    out=tbv[cls][:, v0:v0 + MW],
                    in0=psO[cls][:, 0:MW],
                    in1=evs[cls][:, v0 + 1:v0 + 1 + MW],
                    op=MAX)
                # m = max(E[v], tb)  (2x: flat bf16 SBUF)
                nc.vector.tensor_tensor(
                    out=mbv[cls][:, v0:v0 + MW],
                    in0=tbv[cls][:, v0:v0 + MW],
                    in1=evs[cls][:, v0:v0 + MW],
                    op=MAX)
                # pooled rows ready: (2p+2)*324 + 321 <= mdone
                mdone = v0 + MW
                pready = min(NPOOL, max(0, (mdone - 969) // 648 + 1))
                if pready - pdone[cls] >= 3 or (pready == NPOOL
                                                and pready > pdone[cls]):
                    emit_vert(cls, pready)
                    for tgt in (4, 8, 10):
                        if pdone[cls] >= tgt and odone[cls] < tgt:
                            emit_out(cls, tgt)
        for cls in range(2):
            if pdone[cls] < NPOOL:
                emit_vert(cls, NPOOL)
            if odone[cls] < NPOOL:
                emit_out(cls, NPOOL)

    nc.finalize()
    return nc


_NC_CACHE = {}


def _get_nc(with_bias=False):
    if with_bias not in _NC_CACHE:
        _NC_CACHE[with_bias] = _build_bass(with_bias)
    return _NC_CACHE[with_bias]


def build_in_maps(update_location, feature_map, weight, gamma, beta,
                  running_mean, running_var):
    fm = np.asarray(feature_map, np.float32)
    loc = np.asarray(update_location).astype(np.int64)
    wt_ = np.asarray(weight, np.float32)
    gam = np.asarray(gamma, np.float32)
    bet = np.asarray(beta, np.float32)
    mu = np.asarray(running_mean, np.float32)
    var = np.asarray(running_var, np.float32)

    inv = gam / np.sqrt(var + BN_EPS)
    wf = wt_ * inv[None, None, None, :]          # [7,7,3,64]
    bias = bet - mu * inv                        # [64]

    # fm_pad with extra bottom rows so group1 of core 7 stays in bounds
    fmp = np.zeros((H + 2 * PAD + 2, W + 2 * PAD, CIN), np.float32)
    fmp[PAD:PAD + H, PAD:PAD + W] = fm

    # inactive flag per output pixel; cols >= 640 and rows >= 640 inactive
    flag = np.ones((H + 2, W + 6), np.float32)
    flag[loc[:, 0], loc[:, 1]] = 0.0
    flag[:, W:] = 1.0
    flag[H:, :] = 1.0

    # weight rows [128, 192]: pass p block = W'[i, j'+3p, ch, :]
    wrows = np.zeros((64, 192), np.float32)
    for jp in range(3):
        for i in range(KK):
            for ch in range(CIN):
                row = jp * 21 + i * 3 + ch
                for p in range(3):
                    j = jp + 3 * p
                    if j <= 6:
                        wrows[row, 64 * p:64 * p + 64] = wf[i, j, ch]
    wrows[63, 0:64] = NEG
    wt128 = np.concatenate([wrows, wrows], axis=0).astype(ml_dtypes.bfloat16)
    with_bias = bool(np.any(bias != 0.0))
    bias128 = np.concatenate([bias, bias]).reshape(128, 1).astype(np.float32)

    in_maps = []
    for k in range(NCORES):
        tblk = np.zeros((128, GPAD), ml_dtypes.bfloat16)
        for g in range(2):
            r0 = 80 * k + 40 * g
            S = np.zeros((64, GROWS, WP), np.float32)
            for jp in range(3):
                for i in range(KK):
                    for ch in range(CIN):
                        S[jp * 21 + i * 3 + ch, :, 0:W + 2 * PAD - jp] = \
                            fmp[r0 + i:r0 + i + GROWS, jp:, ch]
            S[63, :, 0:W + 6] = flag[r0:r0 + GROWS, :]
            tblk[64 * g:64 * g + 64, 0:GCOLS] = \
                S.reshape(64, GCOLS).astype(ml_dtypes.bfloat16)
        m = {"tbl": tblk, "wt": wt128}
        if with_bias:
            m["bias"] = bias128
        in_maps.append(m)
    return in_maps, with_bias


def kernel(update_location, feature_map, weight, gamma, beta, running_mean,
           running_var):
    from concourse.bass_utils import run_bass_kernel_spmd

    in_maps, with_bias = build_in_maps(
        update_location, feature_map, weight, gamma, beta, running_mean,
        running_var)
    nc = _get_nc(with_bias)
    res = run_bass_kernel_spmd(nc, in_maps, core_ids=list(range(NCORES)))
    # per-core out [128, 6400] bf16: [part, cls*3200 + p*320 + q];
    # partitions 0-63 = channels of the lower sub-band, 64-127 = upper.
    out = np.zeros((NCORES * 40, 320, COUT), np.float32)
    for k in range(NCORES):
        o = np.asarray(res.results[k]["out"], dtype=np.float32)
        o = o.reshape(2, 64, 2, NPOOL, 320)      # [half, ch, cls, p, q]
        for cls in range(2):
            for half in range(2):
                band = 2 * cls + half
                r = 40 * k + 10 * band
                out[r:r + NPOOL] = o[half, :, cls].transpose(1, 2, 0)
    return np.ascontiguousarray(out[:319, :319, :]).astype(np.float32)
